# revision 15
# baseline (speedup 1.0000x reference)
"""Trainium2 Bass kernel for nn_CausalSelfAttention_30365418782934.

Sharding: 8 cores = 4 batches x 2 head-groups (tensor-parallel over heads,
data-parallel over batch).  Each core computes, for its batch b and its 8
heads: qkv projection, value-residual mix, per-head RMS norm, rotary
(folded into the weights on the host -- it is a per-head linear map of the
d axis), causal attention, and its partial contribution to the output
projection.  The host sums the two partial outputs per batch and adds
b_proj.

Device layouts:
  x^T resident in SBUF as [128c, 16kt, 1024t]   (channel-major)
  Q,K produced channel-major [128d, 1024t] per head (GEMM with W as lhsT)
  V produced token-major [128t, mt, 1024(h d)]  (GEMM with x^T as lhsT)
  attention computed transposed: S^T[k,q] tiles, exp on ACT (K's rms scale
  folded into the activation's per-partition scale), softmax denominator
  via ones-vector matmul, O^T[d,t] accumulated in PSUM = exactly the lhsT
  layout the output projection needs.

All matmuls run in float32r (fp32 operands truncated to fp22 in the PE,
full 1 cycle/row rate at N>=256; accumulation in fp32 PSUM).  Row-shaped
[1,512] intermediates (sumsq, softmax denominator) are transposed to
[128,n] via small DMAs before sqrt/reciprocal -- per-partition-serial DVE
ops on rows cost ~3.3us each, transposed they cost ~0.2us.
"""

import sys

if "/opt/trn_rl_repo" not in sys.path:
    sys.path.insert(0, "/opt/trn_rl_repo")

import numpy as np

B, T, C, H, D = 4, 1024, 2048, 16, 128
HPC = 8          # heads per core
NKT = C // 128   # 16 contraction tiles
EPS = 1e-6
ROPE_BASE = 10000.0
NEG = -1.0e30

_CACHE = {}


# ----------------------------------------------------------------------------
# device program (SPMD, identical on all 8 cores)
# ----------------------------------------------------------------------------
def build_program():
    from contextlib import ExitStack

    import concourse.bass as bass
    import concourse.mybir as mybir
    import concourse.tile as tile
    from concourse import bacc

    f32 = mybir.dt.float32
    f32r = mybir.dt.float32r
    AF = mybir.ActivationFunctionType

    nc = bacc.Bacc(None, target_bir_lowering=False)

    xT = nc.declare_dram_parameter("xT", [NKT, 128, T], f32r, isOutput=False)
    Wqk = nc.declare_dram_parameter("Wqk", [2 * HPC, NKT, 128, 128], f32r, isOutput=False)
    bqk = nc.declare_dram_parameter("bqk", [128, 2 * HPC], f32, isOutput=False)
    Wv = nc.declare_dram_parameter("Wv", [NKT, 128, HPC * D], f32r, isOutput=False)
    v1m = nc.declare_dram_parameter("v1m", [2, 8, 128, 512], f32, isOutput=False)
    Wp = nc.declare_dram_parameter("Wp", [HPC, 128, C], f32r, isOutput=False)
    bf16 = mybir.dt.bfloat16
    mneg = nc.declare_dram_parameter("mneg", [4, 128, 512], bf16, isOutput=False)
    outp = nc.declare_dram_parameter("outp", [8, 128, C], f32, isOutput=True)

    with tile.TileContext(nc) as tc, ExitStack() as ctx, \
            nc.allow_low_precision(reason="fp32r matmul operands (fp22 in PE)"):
        consts = ctx.enter_context(tc.tile_pool(name="consts", bufs=1))
        ones_col0 = consts.tile([128, 1], f32)
        nc.vector.memset(ones_col0, 1.0)
        ones_col = consts.tile([128, 1], f32r)
        nc.vector.tensor_copy(ones_col, ones_col0)
        ones_row0 = consts.tile([1, 128], f32)
        nc.vector.memset(ones_row0, 1.0)
        ones_row = consts.tile([1, 128], f32r)
        nc.vector.tensor_copy(ones_row, ones_row0)
        epsq = consts.tile([1, 1], f32)
        nc.vector.memset(epsq, float(D) * EPS)
        epsk = consts.tile([1, 1], f32)
        nc.vector.memset(epsk, EPS)
        bqk_sb = consts.tile([128, 2 * HPC], f32)
        nc.sync.dma_start(out=bqk_sb, in_=bqk[:, :])
        mask_sb = consts.tile([128, 4, 512], bf16)
        for o in range(4):
            nc.sync.dma_start(out=mask_sb[:, o, :], in_=mneg[o])

        xpool = ctx.enter_context(tc.tile_pool(name="x", bufs=1))
        xsb = xpool.tile([128, NKT, T], f32r)

        vpool = ctx.enter_context(tc.tile_pool(name="v", bufs=1))
        vsb = vpool.tile([128, 8, HPC * D], f32r)  # [tok128, mt, (h d)]
        opool = ctx.enter_context(tc.tile_pool(name="o", bufs=1))
        osb = opool.tile([128, HPC, T], f32r)      # [d, h, tok]

        # ---------------- phase B: V = x @ Wv' + v1m (token-major) ----------
        with tc.tile_pool(name="wv", bufs=1) as wvp, \
             tc.tile_pool(name="v1t", bufs=2) as v1p, \
             tc.tile_pool(name="vg", bufs=3, space="PSUM") as vgp:
            wv_sb = wvp.tile([128, NKT, T], f32r, tag="wv")
            # interleave x^T and Wv chunk loads so the kt-inner accumulation
            # can start as soon as the first chunks land
            for kt in range(NKT):
                nc.sync.dma_start(out=xsb[:, kt, :], in_=xT[kt])
                nc.gpsimd.dma_start(out=wv_sb[:, kt, :], in_=Wv[kt])
            for nt in range(2):
                for mt in range(8):
                    ps = vgp.tile([128, 512], f32, tag="vg")
                    for kt in range(NKT):
                        nc.tensor.matmul(
                            ps,
                            xsb[:, kt, mt * 128:(mt + 1) * 128],
                            wv_sb[:, kt, nt * 512:(nt + 1) * 512],
                            start=(kt == 0),
                            stop=(kt == NKT - 1),
                        )
                    v1t = v1p.tile([128, 512], f32, tag="v1t")
                    nc.sync.dma_start(out=v1t, in_=v1m[nt, mt])
                    nc.vector.tensor_add(
                        vsb[:, mt, nt * 512:(nt + 1) * 512], ps, v1t
                    )

        # ------------- phases C+D, cross-head pipelined ---------------------
        # PE executes its stream in order, so we emit head h+1's GEMM
        # matmuls BEFORE head h's attention matmuls: the long rs/broadcast
        # chains of head h (ACT/DVE/DMA) resolve while the PE chews through
        # head h+1's GEMM, and the attention matmuls then flow stall-free.
        with tc.tile_pool(name="wq", bufs=3) as wqp, \
             tc.tile_pool(name="qh", bufs=2) as qhp, \
             tc.tile_pool(name="kh", bufs=2) as khp, \
             tc.tile_pool(name="rs", bufs=2) as rsp, \
             tc.tile_pool(name="sq", bufs=2) as sqp, \
             tc.tile_pool(name="qb", bufs=3) as qbp, \
             tc.tile_pool(name="pt", bufs=3) as ptp, \
             tc.tile_pool(name="scr", bufs=2, space="DRAM") as scrp, \
             tc.tile_pool(name="qkps", bufs=2, space="PSUM") as qkps, \
             tc.tile_pool(name="rowps", bufs=2, space="PSUM") as rowps, \
             tc.tile_pool(name="sps", bufs=2, space="PSUM") as sps, \
             tc.tile_pool(name="ops", bufs=2, space="PSUM") as opsp:

            def gemm_phase(hh):
                st = {}
                q_t = qhp.tile([128, T], f32r, tag="qh", name=f"q_t{hh}")
                k_t = khp.tile([128, T], f32r, tag="kh", name=f"k_t{hh}")
                ssqT_q = rsp.tile([128, 8], f32, tag="ssqT_q", name=f"ssqTq{hh}")
                ssqT_k = rsp.tile([128, 8], f32, tag="ssqT_k", name=f"ssqTk{hh}")
                rsqT = rsp.tile([128, 8], f32, tag="rsqT", name=f"rsqT{hh}")
                rskT = rsp.tile([128, 8], f32, tag="rskT", name=f"rskT{hh}")
                sc_q = scrp.tile([1, T], f32, tag="sc_q", name=f"scq{hh}")
                sc_k = scrp.tile([1, T], f32, tag="sc_k", name=f"sck{hh}")
                sc_rq = scrp.tile([1, T], f32, tag="sc_rq", name=f"scrq{hh}")
                qbs = {}
                psums = {}
                # 1) the 64 GEMM matmuls, PE-dense
                for is_k in (0, 1):
                    j = 2 * hh + is_k
                    wq_sb = wqp.tile([128, NKT, 128], f32r, tag="wq",
                                     name=f"wq{hh}_{is_k}")
                    nc.sync.dma_start(
                        out=wq_sb, in_=Wqk[j].rearrange("kt p c -> p kt c")
                    )
                    for nt in range(2):
                        ps = qkps.tile([128, 512], f32, tag="qkps",
                                       name=f"qkps{hh}_{is_k}_{nt}")
                        for kt in range(NKT):
                            nc.tensor.matmul(
                                ps,
                                wq_sb[:, kt, :],
                                xsb[:, kt, nt * 512:(nt + 1) * 512],
                                start=(kt == 0),
                                stop=(kt == NKT - 1),
                            )
                        psums[(is_k, nt)] = ps
                # 2) epilogues: bias add, square, sumsq matmul, sqrt row,
                #    bounce to DRAM transposed
                for is_k in (0, 1):
                    bias = bqk_sb[:, 2 * hh + is_k:2 * hh + is_k + 1]
                    for nt in range(2):
                        ps = psums[(is_k, nt)]
                        if is_k:
                            qb = k_t[:, nt * 512:(nt + 1) * 512]
                            nc.vector.tensor_scalar_add(qb, ps, bias)
                            qbf = qb.bitcast(f32)
                        else:
                            qbf = qbp.tile([128, 512], f32, tag="qb",
                                           name=f"qb{hh}_{nt}")
                            nc.vector.tensor_scalar_add(qbf, ps, bias)
                            qbs[nt] = qbf
                        sq = sqp.tile([128, 512], f32r, tag="sq",
                                      name=f"sq{hh}_{is_k}_{nt}")
                        nc.vector.tensor_mul(sq, qbf, qbf)
                        ssq = rowps.tile([1, 512], f32, tag="rowps",
                                         name=f"ssq{hh}_{is_k}_{nt}")
                        nc.tensor.matmul(ssq, ones_col, sq, start=True, stop=True)
                        srow = qbp.tile([1, 512], f32, tag="srow",
                                        name=f"srow{hh}_{is_k}_{nt}")
                        if is_k:
                            nc.scalar.activation(srow, ssq, AF.Sqrt,
                                                 bias=epsk, scale=1.0 / D)
                            nc.scalar.dma_start(out=sc_k[:, nt * 512:(nt + 1) * 512], in_=srow)
                        else:
                            nc.scalar.activation(srow, ssq, AF.Sqrt,
                                                 bias=epsq, scale=1.0)
                            nc.scalar.dma_start(out=sc_q[:, nt * 512:(nt + 1) * 512], in_=srow)
                # 3) rs chains in [128,8] layout via DRAM-transposed reads
                nc.scalar.dma_start(
                    out=ssqT_q,
                    in_=sc_q.rearrange("a (c p) -> a p c", p=128),
                )
                nc.vector.reciprocal(rsqT, ssqT_q)
                nc.scalar.dma_start(
                    out=ssqT_k,
                    in_=sc_k.rearrange("a (c p) -> a p c", p=128),
                )
                nc.vector.reciprocal(rskT, ssqT_k)
                # rs_q back to a DRAM row, then partition-broadcast reads
                nc.scalar.dma_start(
                    out=sc_rq.rearrange("a (c p) -> a p c", p=128), in_=rsqT
                )
                for nt in range(2):
                    bcs = sqp.tile([128, 512], f32, tag="bcs",
                                   name=f"bcs{hh}_{nt}")
                    half = sc_rq[:, nt * 512:(nt + 1) * 512]
                    bcast_ap = bass.AP(tensor=half.tensor, offset=half.offset,
                                       ap=[[0, 128]] + half.ap[1:])
                    nc.scalar.dma_start(out=bcs, in_=bcast_ap)
                    nc.vector.tensor_mul(
                        q_t[:, nt * 512:(nt + 1) * 512], qbs[nt], bcs
                    )
                st["q_t"], st["k_t"], st["rskT"] = q_t, k_t, rskT
                return st

            def attn_phase(hh, st):
                q_t, k_t, rskT = st["q_t"], st["k_t"], st["rskT"]
                for qt in range(2):
                    kts = list(range(4)) if qt == 0 else list(range(8))
                    nk = len(kts)
                    o_ps = opsp.tile([128, 512], f32, tag="ops",
                                     name=f"ops{hh}_{qt}")
                    aden = rowps.tile([1, 512], f32, tag="rowps",
                                      name=f"aden{hh}_{qt}")
                    s_tiles = [None] * nk
                    p_tiles = [None] * nk

                    def emit_s(i):
                        kt = kts[i]
                        s_ps = sps.tile([128, 512], f32, tag="sps",
                                        name=f"sps{hh}_{qt}_{i}")
                        nc.tensor.matmul(
                            s_ps,
                            k_t[:, kt * 128:(kt + 1) * 128],
                            q_t[:, qt * 512:(qt + 1) * 512],
                            start=True, stop=True,
                        )
                        off = kt * 128 - qt * 512
                        if off >= 0:
                            nc.vector.tensor_add(
                                s_ps, s_ps, mask_sb[:, off // 128, :]
                            )
                        s_tiles[i] = s_ps

                    def emit_exp(i):
                        kt = kts[i]
                        pt = ptp.tile([128, 512], f32r, tag="pt",
                                      name=f"pt{hh}_{qt}_{i}")
                        nc.scalar.activation(
                            pt, s_tiles[i], AF.Exp, bias=0.0,
                            scale=rskT[:, kt:kt + 1],
                        )
                        p_tiles[i] = pt

                    emit_s(0)
                    if nk > 1:
                        emit_s(1)
                    emit_exp(0)
                    for i in range(nk):
                        if i + 2 < nk:
                            emit_s(i + 2)
                        if i + 1 < nk:
                            emit_exp(i + 1)
                        pt = p_tiles[i]
                        first, last = (i == 0), (i == nk - 1)
                        nc.tensor.matmul(
                            aden, ones_col, pt, start=first, stop=last
                        )
                        nc.tensor.matmul(
                            o_ps,
                            vsb[:, kts[i], hh * 128:(hh + 1) * 128],
                            pt,
                            start=first, stop=last,
                        )
                    # normalize: 1/den via DRAM-transposed bounce
                    adrow = qbp.tile([1, 512], f32, tag="srow",
                                     name=f"adrow{hh}_{qt}")
                    nc.scalar.copy(adrow, aden)
                    sc_d = scrp.tile([1, 512], f32, tag="sc_d",
                                     name=f"scd{hh}_{qt}")
                    nc.gpsimd.dma_start(out=sc_d[:, :], in_=adrow)
                    adenT = rsp.tile([128, 4], f32, tag="adenT",
                                     name=f"adenT{hh}_{qt}")
                    nc.gpsimd.dma_start(
                        out=adenT,
                        in_=sc_d.rearrange("a (c p) -> a p c", p=128),
                    )
                    recT = rsp.tile([128, 4], f32, tag="recT",
                                    name=f"recT{hh}_{qt}")
                    nc.vector.reciprocal(recT, adenT)
                    sc_r = scrp.tile([1, 512], f32, tag="sc_r",
                                     name=f"scr{hh}_{qt}")
                    nc.gpsimd.dma_start(
                        out=sc_r.rearrange("a (c p) -> a p c", p=128), in_=recT
                    )
                    abcs = sqp.tile([128, 512], f32, tag="bcs",
                                    name=f"abcs{hh}_{qt}")
                    bcast_ap = bass.AP(tensor=sc_r.tensor, offset=sc_r.offset,
                                       ap=[[0, 128]] + sc_r.ap[1:])
                    nc.gpsimd.dma_start(out=abcs, in_=bcast_ap)
                    nc.vector.tensor_mul(
                        osb[:, hh, qt * 512:(qt + 1) * 512], o_ps, abcs
                    )

            stash = gemm_phase(0)
            for hh in range(HPC):
                nxt = gemm_phase(hh + 1) if hh + 1 < HPC else None
                attn_phase(hh, stash)
                stash = nxt

        # ---------------- phase E: out_partial = attn @ Wp ------------------
        with tc.tile_pool(name="wp", bufs=2) as wpp, \
             tc.tile_pool(name="outs", bufs=3) as outsp, \
             tc.tile_pool(name="pps", bufs=3, space="PSUM") as pps:
            for nt in range(4):
                wp_sb = wpp.tile([128, HPC, 512], f32r, tag="wp")
                for hb in range(HPC):
                    nc.sync.dma_start(
                        out=wp_sb[:, hb, :],
                        in_=Wp[hb][:, nt * 512:(nt + 1) * 512],
                    )
                for mt in range(8):
                    ps = pps.tile([128, 512], f32, tag="pps")
                    for hb in range(HPC):
                        nc.tensor.matmul(
                            ps,
                            osb[:, hb, mt * 128:(mt + 1) * 128],
                            wp_sb[:, hb, :],
                            start=(hb == 0),
                            stop=(hb == HPC - 1),
                        )
                    ot = outsp.tile([128, 512], f32, tag="outs")
                    nc.scalar.copy(ot, ps)
                    nc.sync.dma_start(
                        out=outp[mt][:, nt * 512:(nt + 1) * 512], in_=ot
                    )

    nc.finalize()
    return nc


# ----------------------------------------------------------------------------
# host-side sharding
# ----------------------------------------------------------------------------
def _rot_mat(h):
    inv_freq = 1.0 / (ROPE_BASE ** (np.arange(0, D, 2, dtype=np.float64) / D))
    ang = h * inv_freq  # (64,)
    c, s = np.cos(ang), np.sin(ang)
    R = np.zeros((D, D), dtype=np.float64)
    i = np.arange(64)
    R[i, i] = c
    R[i, i + 64] = s
    R[i + 64, i] = -s
    R[i + 64, i + 64] = c
    return R


def make_in_maps(x, v1, W_qkv, b_qkv, W_proj, b_proj, lamb):
    x = np.asarray(x, np.float32)
    v1 = np.asarray(v1, np.float32)
    W_qkv = np.asarray(W_qkv, np.float32)
    b_qkv = np.asarray(b_qkv, np.float32)
    W_proj = np.asarray(W_proj, np.float32)
    b_proj = np.asarray(b_proj, np.float32)
    lam = float(np.asarray(lamb))

    W_q, W_k, W_v = W_qkv[:, :C], W_qkv[:, C:2 * C], W_qkv[:, 2 * C:]
    b_q, b_k, b_v = b_qkv[:C], b_qkv[C:2 * C], b_qkv[2 * C:]

    # fold rotary into the q/k weights+biases (R_h orthogonal)
    Rs = [_rot_mat(h) for h in range(H)]
    Wq_rot = np.empty_like(W_q)
    Wk_rot = np.empty_like(W_k)
    bq_rot = np.empty_like(b_q)
    bk_rot = np.empty_like(b_k)
    for h in range(H):
        sl = slice(h * D, (h + 1) * D)
        R = Rs[h]
        Wq_rot[:, sl] = (W_q[:, sl].astype(np.float64) @ R.T).astype(np.float32)
        Wk_rot[:, sl] = (W_k[:, sl].astype(np.float64) @ R.T).astype(np.float32)
        bq_rot[sl] = (R @ b_q[sl].astype(np.float64)).astype(np.float32)
        bk_rot[sl] = (R @ b_k[sl].astype(np.float64)).astype(np.float32)

    # causal additive masks for the diagonal-band blocks, S^T layout [k, q]
    mneg = np.zeros((4, 128, 512), np.float32)
    rr = np.arange(128)[:, None]
    cc = np.arange(512)[None, :]
    for o in range(4):
        mneg[o] = np.where(cc >= rr + o * 128, 0.0, NEG).astype(np.float32)
    import ml_dtypes
    mneg = mneg.astype(ml_dtypes.bfloat16)

    in_maps = []
    for core in range(8):
        b = core // 2
        g = core % 2
        heads = range(g * HPC, (g + 1) * HPC)

        xTc = np.ascontiguousarray(x[b].T).reshape(NKT, 128, T)

        Wqk_c = np.empty((2 * HPC, NKT, 128, 128), np.float32)
        bqk_c = np.empty((128, 2 * HPC), np.float32)
        for hh, h in enumerate(heads):
            sl = slice(h * D, (h + 1) * D)
            Wqk_c[2 * hh] = Wq_rot[:, sl].reshape(NKT, 128, 128)
            Wqk_c[2 * hh + 1] = Wk_rot[:, sl].reshape(NKT, 128, 128)
            bqk_c[:, 2 * hh] = bq_rot[sl]
            bqk_c[:, 2 * hh + 1] = bk_rot[sl]

        gsl = slice(g * HPC * D, (g + 1) * HPC * D)
        Wv_c = ((1.0 - lam) * W_v[:, gsl]).reshape(NKT, 128, HPC * D)
        v1_c = lam * v1[b, :, g * HPC:(g + 1) * HPC, :].reshape(T, HPC * D) \
            + (1.0 - lam) * b_v[gsl][None, :]
        v1m_c = np.ascontiguousarray(
            v1_c.reshape(8, 128, 2, 512).transpose(2, 0, 1, 3)
        )
        Wp_c = W_proj[gsl, :].reshape(HPC, 128, C)

        in_maps.append({
            "xT": np.ascontiguousarray(xTc),
            "Wqk": np.ascontiguousarray(Wqk_c),
            "bqk": np.ascontiguousarray(bqk_c),
            "Wv": np.ascontiguousarray(Wv_c),
            "v1m": v1m_c,
            "Wp": np.ascontiguousarray(Wp_c),
            "mneg": mneg,
        })
    return in_maps


def gather(results, b_proj):
    b_proj = np.asarray(b_proj, np.float32)
    out = np.zeros((B, T, C), np.float32)
    for core, res in enumerate(results):
        out[core // 2] += res["outp"].reshape(T, C)
    out += b_proj[None, None, :]
    return out


# ----------------------------------------------------------------------------
# public entry point
# ----------------------------------------------------------------------------
def kernel(x, v1, W_qkv, b_qkv, W_proj, b_proj, lamb):
    from concourse.bass_utils import run_bass_kernel_spmd

    if "nc" not in _CACHE:
        _CACHE["nc"] = build_program()
    nc = _CACHE["nc"]

    in_maps = make_in_maps(x, v1, W_qkv, b_qkv, W_proj, b_proj, lamb)
    res = run_bass_kernel_spmd(nc, in_maps, list(range(8)))
    out = gather(res.results, b_proj)
    return out, np.asarray(v1, np.float32)


# revision 16
# speedup vs baseline: 1.4972x; 1.4972x over previous
"""Trainium2 Bass kernel for nn_CausalSelfAttention_30365418782934.

Sharding: 8 cores = 4 batches x 2 head-groups (tensor-parallel over heads,
data-parallel over batch).  Each core computes, for its batch b and its 8
heads: qkv projection, value-residual mix, per-head RMS norm, rotary
(folded into the weights on the host -- it is a per-head linear map of the
d axis), causal attention, and its partial contribution to the output
projection.  The host sums the two partial outputs per batch and adds
b_proj.

Device layouts:
  x^T resident in SBUF as [128c, 16kt, 1024t]   (channel-major)
  Q,K produced channel-major [128d, 1024t] per head (GEMM with W as lhsT)
  V produced token-major [128t, mt, 1024(h d)]  (GEMM with x^T as lhsT)
  attention computed transposed: S^T[k,q] tiles, exp on ACT (K's rms scale
  folded into the activation's per-partition scale), softmax denominator
  via ones-vector matmul, O^T[d,t] accumulated in PSUM = exactly the lhsT
  layout the output projection needs.

All matmuls run in float32r (fp32 operands truncated to fp22 in the PE,
full 1 cycle/row rate at N>=256; accumulation in fp32 PSUM).  Row-shaped
[1,512] intermediates (sumsq, softmax denominator) are transposed to
[128,n] via small DMAs before sqrt/reciprocal -- per-partition-serial DVE
ops on rows cost ~3.3us each, transposed they cost ~0.2us.
"""

import sys

if "/opt/trn_rl_repo" not in sys.path:
    sys.path.insert(0, "/opt/trn_rl_repo")

import numpy as np

B, T, C, H, D = 4, 1024, 2048, 16, 128
HPC = 8          # heads per core
NKT = C // 128   # 16 contraction tiles
EPS = 1e-6
ROPE_BASE = 10000.0
NEG = -1.0e30

_CACHE = {}


# ----------------------------------------------------------------------------
# device program (SPMD, identical on all 8 cores)
# ----------------------------------------------------------------------------
def build_program():
    from contextlib import ExitStack

    import concourse.bass as bass
    import concourse.mybir as mybir
    import concourse.tile as tile
    from concourse import bacc

    f32 = mybir.dt.float32
    f32r = mybir.dt.float32r
    AF = mybir.ActivationFunctionType

    nc = bacc.Bacc(None, target_bir_lowering=False)

    xT = nc.declare_dram_parameter("xT", [NKT, 128, T], f32r, isOutput=False)
    Wqk = nc.declare_dram_parameter("Wqk", [2 * HPC, NKT, 128, 128], f32r, isOutput=False)
    bqk = nc.declare_dram_parameter("bqk", [128, 2 * HPC], f32, isOutput=False)
    Wv = nc.declare_dram_parameter("Wv", [NKT, 128, HPC * D], f32r, isOutput=False)
    v1m = nc.declare_dram_parameter("v1m", [2, 8, 128, 512], f32, isOutput=False)
    Wp = nc.declare_dram_parameter("Wp", [HPC, 128, C], f32r, isOutput=False)
    bf16 = mybir.dt.bfloat16
    mneg = nc.declare_dram_parameter("mneg", [4, 128, 512], bf16, isOutput=False)
    outp = nc.declare_dram_parameter("outp", [8, 128, C], f32, isOutput=True)

    with tile.TileContext(nc) as tc, ExitStack() as ctx, \
            nc.allow_low_precision(reason="fp32r matmul operands (fp22 in PE)"):
        consts = ctx.enter_context(tc.tile_pool(name="consts", bufs=1))
        ones_col0 = consts.tile([128, 1], f32)
        nc.vector.memset(ones_col0, 1.0)
        ones_col = consts.tile([128, 1], f32r)
        nc.vector.tensor_copy(ones_col, ones_col0)
        ones_row0 = consts.tile([1, 128], f32)
        nc.vector.memset(ones_row0, 1.0)
        ones_row = consts.tile([1, 128], f32r)
        nc.vector.tensor_copy(ones_row, ones_row0)
        epsq = consts.tile([1, 1], f32)
        nc.vector.memset(epsq, float(D) * EPS)
        epsk = consts.tile([1, 1], f32)
        nc.vector.memset(epsk, EPS)
        bqk_sb = consts.tile([128, 2 * HPC], f32)
        nc.sync.dma_start(out=bqk_sb, in_=bqk[:, :])
        mask_sb = consts.tile([128, 4, 512], bf16)
        for o in range(4):
            nc.sync.dma_start(out=mask_sb[:, o, :], in_=mneg[o])

        xpool = ctx.enter_context(tc.tile_pool(name="x", bufs=1))
        xsb = xpool.tile([128, NKT, T], f32r)

        vpool = ctx.enter_context(tc.tile_pool(name="v", bufs=1))
        vsb = vpool.tile([128, 8, HPC * D], f32r)  # [tok128, mt, (h d)]
        opool = ctx.enter_context(tc.tile_pool(name="o", bufs=1))
        osb = opool.tile([128, HPC, T], f32r)      # [d, h, tok]

        # ---------------- phase B: V = x @ Wv' + v1m (token-major) ----------
        with tc.tile_pool(name="wv", bufs=1) as wvp, \
             tc.tile_pool(name="v1t", bufs=2) as v1p, \
             tc.tile_pool(name="vg", bufs=3, space="PSUM") as vgp:
            wv_sb = wvp.tile([128, NKT, T], f32r, tag="wv")
            # interleave x^T and Wv chunk loads so the kt-inner accumulation
            # can start as soon as the first chunks land
            for kt in range(NKT):
                nc.sync.dma_start(out=xsb[:, kt, :], in_=xT[kt])
                nc.gpsimd.dma_start(out=wv_sb[:, kt, :], in_=Wv[kt])
            for nt in range(2):
                for mt in range(8):
                    ps = vgp.tile([128, 512], f32, tag="vg")
                    for kt in range(NKT):
                        nc.tensor.matmul(
                            ps,
                            xsb[:, kt, mt * 128:(mt + 1) * 128],
                            wv_sb[:, kt, nt * 512:(nt + 1) * 512],
                            start=(kt == 0),
                            stop=(kt == NKT - 1),
                        )
                    v1t = v1p.tile([128, 512], f32, tag="v1t")
                    nc.sync.dma_start(out=v1t, in_=v1m[nt, mt])
                    nc.vector.tensor_add(
                        vsb[:, mt, nt * 512:(nt + 1) * 512], ps, v1t
                    )

        # ------------- phases C+D, cross-head pipelined ---------------------
        # PE executes its stream in order, so we emit head h+1's GEMM
        # matmuls BEFORE head h's attention matmuls: head h's normalization
        # chains (ACT/DVE) resolve while the PE chews through head h+1's
        # GEMM, and the attention matmuls then flow stall-free.
        # All row-shaped math (rms scale, softmax denominator) stays on the
        # engines: Abs_reciprocal_sqrt on ACT (measured 4e-5 max rel err)
        # plus a ones-row matmul to broadcast rows across partitions.
        with tc.tile_pool(name="wq", bufs=3) as wqp, \
             tc.tile_pool(name="qh", bufs=2) as qhp, \
             tc.tile_pool(name="kh", bufs=2) as khp, \
             tc.tile_pool(name="sq", bufs=3) as sqp, \
             tc.tile_pool(name="rowsb", bufs=3) as rowsbp, \
             tc.tile_pool(name="pt", bufs=4) as ptp, \
             tc.tile_pool(name="qkps", bufs=2, space="PSUM") as qkps, \
             tc.tile_pool(name="rowps", bufs=2, space="PSUM") as rowps, \
             tc.tile_pool(name="bcps", bufs=1, space="PSUM") as bcps, \
             tc.tile_pool(name="sps", bufs=2, space="PSUM") as sps, \
             tc.tile_pool(name="ops", bufs=1, space="PSUM") as opsp:

            def gemm_phase(hh):
                st = {}
                q_t = qhp.tile([128, T], f32r, tag="qh", name=f"q_t{hh}")
                k_t = khp.tile([128, T], f32r, tag="kh", name=f"k_t{hh}")
                # 1) the 64 GEMM matmuls, PE-dense
                psums = {}
                for is_k in (0, 1):
                    j = 2 * hh + is_k
                    wq_sb = wqp.tile([128, NKT, 128], f32r, tag="wq",
                                     name=f"wq{hh}_{is_k}")
                    nc.sync.dma_start(
                        out=wq_sb, in_=Wqk[j].rearrange("kt p c -> p kt c")
                    )
                    for nt in range(2):
                        ps = qkps.tile([128, 512], f32, tag="qkps",
                                       name=f"qkps{hh}_{is_k}_{nt}")
                        for kt in range(NKT):
                            nc.tensor.matmul(
                                ps,
                                wq_sb[:, kt, :],
                                xsb[:, kt, nt * 512:(nt + 1) * 512],
                                start=(kt == 0),
                                stop=(kt == NKT - 1),
                            )
                        psums[(is_k, nt)] = ps
                # 2) epilogues: bias add, square, sumsq matmul, rsqrt row
                rows = {}
                for is_k in (0, 1):
                    bias = bqk_sb[:, 2 * hh + is_k:2 * hh + is_k + 1]
                    dst = k_t if is_k else q_t
                    for nt in range(2):
                        ps = psums[(is_k, nt)]
                        sl = dst[:, nt * 512:(nt + 1) * 512]
                        nc.vector.tensor_scalar_add(sl, ps, bias)
                        sq = sqp.tile([128, 512], f32r, tag="sq",
                                      name=f"sq{hh}_{is_k}_{nt}")
                        slf = sl.bitcast(f32)
                        nc.vector.tensor_mul(sq, slf, slf)
                        ssq = rowps.tile([1, 512], f32, tag="rowps",
                                         name=f"ssq{hh}_{is_k}_{nt}")
                        nc.tensor.matmul(ssq, ones_col, sq, start=True, stop=True)
                        rs_row = rowsbp.tile([1, 512], f32r, tag="rs_row",
                                             name=f"rsrow{hh}_{is_k}_{nt}")
                        if is_k:
                            nc.scalar.activation(rs_row, ssq, AF.Abs_reciprocal_sqrt,
                                                 bias=epsk, scale=1.0 / D)
                        else:
                            nc.scalar.activation(rs_row, ssq, AF.Abs_reciprocal_sqrt,
                                                 bias=epsq, scale=1.0)
                        rows[(is_k, nt)] = rs_row
                # 3) broadcast rows via ones-column matmul, scale in place
                for is_k in (0, 1):
                    dst = k_t if is_k else q_t
                    for nt in range(2):
                        bc = bcps.tile([128, 512], f32, tag="bc",
                                       name=f"bc{hh}_{is_k}_{nt}")
                        nc.tensor.matmul(bc, ones_row, rows[(is_k, nt)],
                                         start=True, stop=True)
                        sl = dst[:, nt * 512:(nt + 1) * 512]
                        nc.vector.tensor_mul(sl, bc, sl.bitcast(f32))
                st["q_t"], st["k_t"] = q_t, k_t
                return st

            def attn_phase(hh, st):
                q_t, k_t = st["q_t"], st["k_t"]
                for qt in range(2):
                    kts = list(range(4)) if qt == 0 else list(range(8))
                    nk = len(kts)
                    o_ps = opsp.tile([128, 512], f32, tag="ops",
                                     name=f"ops{hh}_{qt}")
                    aden = rowps.tile([1, 512], f32, tag="rowps",
                                      name=f"aden{hh}_{qt}")
                    s_tiles = [None] * nk
                    p_tiles = [None] * nk

                    def emit_s(i):
                        kt = kts[i]
                        s_ps = sps.tile([128, 512], f32, tag="sps",
                                        name=f"sps{hh}_{qt}_{i}")
                        nc.tensor.matmul(
                            s_ps,
                            k_t[:, kt * 128:(kt + 1) * 128],
                            q_t[:, qt * 512:(qt + 1) * 512],
                            start=True, stop=True,
                        )
                        off = kt * 128 - qt * 512
                        if off >= 0:
                            nc.vector.tensor_add(
                                s_ps, s_ps, mask_sb[:, off // 128, :]
                            )
                        s_tiles[i] = s_ps

                    def emit_exp(i):
                        pt = ptp.tile([128, 512], f32r, tag="pt",
                                      name=f"pt{hh}_{qt}_{i}")
                        nc.scalar.activation(pt, s_tiles[i], AF.Exp)
                        p_tiles[i] = pt

                    emit_s(0)
                    if nk > 1:
                        emit_s(1)
                    emit_exp(0)
                    for i in range(nk):
                        if i + 2 < nk:
                            emit_s(i + 2)
                        if i + 1 < nk:
                            emit_exp(i + 1)
                        pt = p_tiles[i]
                        first, last = (i == 0), (i == nk - 1)
                        nc.tensor.matmul(
                            aden, ones_col, pt, start=first, stop=last
                        )
                        nc.tensor.matmul(
                            o_ps,
                            vsb[:, kts[i], hh * 128:(hh + 1) * 128],
                            pt,
                            start=first, stop=last,
                        )
                    # normalize: 1/den = Arsqrt(den^2), broadcast, multiply
                    d2row = rowsbp.tile([1, 512], f32, tag="d2row",
                                        name=f"d2row{hh}_{qt}")
                    nc.scalar.activation(d2row, aden, AF.Square)
                    rec_row = rowsbp.tile([1, 512], f32r, tag="rec_row",
                                          name=f"recrow{hh}_{qt}")
                    nc.scalar.activation(rec_row, d2row, AF.Abs_reciprocal_sqrt)
                    abc = bcps.tile([128, 512], f32, tag="bc",
                                    name=f"abc{hh}_{qt}")
                    nc.tensor.matmul(abc, ones_row, rec_row, start=True, stop=True)
                    osl = osb[:, hh, qt * 512:(qt + 1) * 512]
                    nc.scalar.copy(osl, o_ps)
                    nc.vector.tensor_mul(osl, abc, osl.bitcast(f32))

            stash = gemm_phase(0)
            for hh in range(HPC):
                nxt = gemm_phase(hh + 1) if hh + 1 < HPC else None
                attn_phase(hh, stash)
                stash = nxt

        # ---------------- phase E: out_partial = attn @ Wp ------------------
        with tc.tile_pool(name="wp", bufs=2) as wpp, \
             tc.tile_pool(name="outs", bufs=3) as outsp, \
             tc.tile_pool(name="pps", bufs=3, space="PSUM") as pps:
            for nt in range(4):
                wp_sb = wpp.tile([128, HPC, 512], f32r, tag="wp")
                for hb in range(HPC):
                    nc.sync.dma_start(
                        out=wp_sb[:, hb, :],
                        in_=Wp[hb][:, nt * 512:(nt + 1) * 512],
                    )
                for mt in range(8):
                    ps = pps.tile([128, 512], f32, tag="pps")
                    for hb in range(HPC):
                        nc.tensor.matmul(
                            ps,
                            osb[:, hb, mt * 128:(mt + 1) * 128],
                            wp_sb[:, hb, :],
                            start=(hb == 0),
                            stop=(hb == HPC - 1),
                        )
                    ot = outsp.tile([128, 512], f32, tag="outs")
                    nc.scalar.copy(ot, ps)
                    nc.sync.dma_start(
                        out=outp[mt][:, nt * 512:(nt + 1) * 512], in_=ot
                    )

    nc.finalize()
    return nc


# ----------------------------------------------------------------------------
# host-side sharding
# ----------------------------------------------------------------------------
def _rot_mat(h):
    inv_freq = 1.0 / (ROPE_BASE ** (np.arange(0, D, 2, dtype=np.float64) / D))
    ang = h * inv_freq  # (64,)
    c, s = np.cos(ang), np.sin(ang)
    R = np.zeros((D, D), dtype=np.float64)
    i = np.arange(64)
    R[i, i] = c
    R[i, i + 64] = s
    R[i + 64, i] = -s
    R[i + 64, i + 64] = c
    return R


def make_in_maps(x, v1, W_qkv, b_qkv, W_proj, b_proj, lamb):
    x = np.asarray(x, np.float32)
    v1 = np.asarray(v1, np.float32)
    W_qkv = np.asarray(W_qkv, np.float32)
    b_qkv = np.asarray(b_qkv, np.float32)
    W_proj = np.asarray(W_proj, np.float32)
    b_proj = np.asarray(b_proj, np.float32)
    lam = float(np.asarray(lamb))

    W_q, W_k, W_v = W_qkv[:, :C], W_qkv[:, C:2 * C], W_qkv[:, 2 * C:]
    b_q, b_k, b_v = b_qkv[:C], b_qkv[C:2 * C], b_qkv[2 * C:]

    # fold rotary into the q/k weights+biases (R_h orthogonal)
    Rs = [_rot_mat(h) for h in range(H)]
    Wq_rot = np.empty_like(W_q)
    Wk_rot = np.empty_like(W_k)
    bq_rot = np.empty_like(b_q)
    bk_rot = np.empty_like(b_k)
    for h in range(H):
        sl = slice(h * D, (h + 1) * D)
        R = Rs[h]
        Wq_rot[:, sl] = (W_q[:, sl].astype(np.float64) @ R.T).astype(np.float32)
        Wk_rot[:, sl] = (W_k[:, sl].astype(np.float64) @ R.T).astype(np.float32)
        bq_rot[sl] = (R @ b_q[sl].astype(np.float64)).astype(np.float32)
        bk_rot[sl] = (R @ b_k[sl].astype(np.float64)).astype(np.float32)

    # causal additive masks for the diagonal-band blocks, S^T layout [k, q]
    mneg = np.zeros((4, 128, 512), np.float32)
    rr = np.arange(128)[:, None]
    cc = np.arange(512)[None, :]
    for o in range(4):
        mneg[o] = np.where(cc >= rr + o * 128, 0.0, NEG).astype(np.float32)
    import ml_dtypes
    mneg = mneg.astype(ml_dtypes.bfloat16)

    in_maps = []
    for core in range(8):
        b = core // 2
        g = core % 2
        heads = range(g * HPC, (g + 1) * HPC)

        xTc = np.ascontiguousarray(x[b].T).reshape(NKT, 128, T)

        Wqk_c = np.empty((2 * HPC, NKT, 128, 128), np.float32)
        bqk_c = np.empty((128, 2 * HPC), np.float32)
        for hh, h in enumerate(heads):
            sl = slice(h * D, (h + 1) * D)
            Wqk_c[2 * hh] = Wq_rot[:, sl].reshape(NKT, 128, 128)
            Wqk_c[2 * hh + 1] = Wk_rot[:, sl].reshape(NKT, 128, 128)
            bqk_c[:, 2 * hh] = bq_rot[sl]
            bqk_c[:, 2 * hh + 1] = bk_rot[sl]

        gsl = slice(g * HPC * D, (g + 1) * HPC * D)
        Wv_c = ((1.0 - lam) * W_v[:, gsl]).reshape(NKT, 128, HPC * D)
        v1_c = lam * v1[b, :, g * HPC:(g + 1) * HPC, :].reshape(T, HPC * D) \
            + (1.0 - lam) * b_v[gsl][None, :]
        v1m_c = np.ascontiguousarray(
            v1_c.reshape(8, 128, 2, 512).transpose(2, 0, 1, 3)
        )
        Wp_c = W_proj[gsl, :].reshape(HPC, 128, C)

        in_maps.append({
            "xT": np.ascontiguousarray(xTc),
            "Wqk": np.ascontiguousarray(Wqk_c),
            "bqk": np.ascontiguousarray(bqk_c),
            "Wv": np.ascontiguousarray(Wv_c),
            "v1m": v1m_c,
            "Wp": np.ascontiguousarray(Wp_c),
            "mneg": mneg,
        })
    return in_maps


def gather(results, b_proj):
    b_proj = np.asarray(b_proj, np.float32)
    out = np.zeros((B, T, C), np.float32)
    for core, res in enumerate(results):
        out[core // 2] += res["outp"].reshape(T, C)
    out += b_proj[None, None, :]
    return out


# ----------------------------------------------------------------------------
# public entry point
# ----------------------------------------------------------------------------
def kernel(x, v1, W_qkv, b_qkv, W_proj, b_proj, lamb):
    from concourse.bass_utils import run_bass_kernel_spmd

    if "nc" not in _CACHE:
        _CACHE["nc"] = build_program()
    nc = _CACHE["nc"]

    in_maps = make_in_maps(x, v1, W_qkv, b_qkv, W_proj, b_proj, lamb)
    res = run_bass_kernel_spmd(nc, in_maps, list(range(8)))
    out = gather(res.results, b_proj)
    return out, np.asarray(v1, np.float32)
